# revision 14
# baseline (speedup 1.0000x reference)
"""GRU-with-peephole (NewGRU) Trainium2 kernel.

B=64, T=512, D=H=512. Data-parallel over batch: 8 cores x 8 batch each.

Phase PRE: a_g[t,b,:] = x[b,t]@W_ig.T + ctx[b,t]@W_pg.T + (b_ig+b_hg+b_pg)
           for g in {r,i,n}, staged to DRAM as [t, b, g, H].
Phase SCAN (sequential over T):
    zr = a_r[t] + h @ W_hr.T          r = sigmoid(zr)
    zi = a_i[t] + h @ W_hi.T          i = sigmoid(zi)
    zn = a_n[t] + (r*h) @ W_hn.T      n = tanh(zn)
    h  = (1-i)*h + i*n
    hist[t] = h  in transposed layout [128(p), 4(c)*8(b)] = h[b, c*128+p]
Host reassembles ys[b,t,h] from hist.

Matmuls are float32r (TF32-like, 1 cyc/row at N=512) with fp32 PSUM
accumulation. The scan keeps h transposed (hT [128, 32]) in a rotating
8-slot SBUF staging tile whose slices are both the matmul inputs and
the DMA-out source; gate matmuls are h-stationary (lhsT = hT chunk
[128,8], rhs = W^T chunk [128,512] moving). Per-step projections are
injected into PSUM via an eye(8) matmul with start=True, which also
initializes the accumulation group.
"""

import sys

if "/opt/trn_rl_repo" not in sys.path:
    sys.path.insert(0, "/opt/trn_rl_repo")

from contextlib import ExitStack

import numpy as np

import concourse.bass as bass
import concourse.bacc as bacc
import concourse.mybir as mybir
import concourse.tile as tile
from concourse.bass_utils import run_bass_kernel_spmd

B, T, D, H = 64, 512, 512, 512
NCORES = 8
BC = B // NCORES  # 8 batch per core
RING = 8          # steps per scan loop body / h+ar staging slots

F32 = mybir.dt.float32
F32R = mybir.dt.float32r
AF = mybir.ActivationFunctionType
ALU = mybir.AluOpType
ds = bass.ds

_CACHE = {}


def r32(ap):
    return ap.bitcast(F32R)


def build_program(t_steps=T):
    """Build the SPMD Bass program for one core (same program on all 8)."""
    assert t_steps % 32 == 0
    nblk = t_steps // 16  # PRE blocks of 16 timesteps
    nc = bacc.Bacc("TRN2", target_bir_lowering=False)

    x_d = nc.declare_dram_parameter("x", [BC, t_steps, D], F32, isOutput=False)
    ctx_d = nc.declare_dram_parameter("ctx", [BC, t_steps, H], F32, isOutput=False)
    h0_d = nc.declare_dram_parameter("h0", [BC, H], F32, isOutput=False)
    wx_d = nc.declare_dram_parameter("wxT", [D, 3 * H], F32R, isOutput=False)
    wp_d = nc.declare_dram_parameter("wpT", [H, 3 * H], F32R, isOutput=False)
    wh_d = nc.declare_dram_parameter("whT", [H, 3 * H], F32R, isOutput=False)
    bias_d = nc.declare_dram_parameter("bias", [1, 3 * H], F32R, isOutput=False)
    eye8_d = nc.declare_dram_parameter("eye8", [8, 8], F32R, isOutput=False)
    eye128_d = nc.declare_dram_parameter("eye128", [128, 128], F32, isOutput=False)
    ones1_d = nc.declare_dram_parameter("ones1", [1, 128], F32R, isOutput=False)
    hist_d = nc.declare_dram_parameter(
        "hist", [t_steps, 128, 4 * BC], F32R, isOutput=True)
    ar_d = nc.dram_tensor("ar_stage", [t_steps, BC, 3, H], F32R)

    with tile.TileContext(nc) as tc, ExitStack() as c0:
        # ---------------- persistent constants / weights ----------------
        consts = c0.enter_context(tc.tile_pool(name="consts", bufs=1))
        eye8 = consts.tile([8, 8], F32R, tag="eye8")
        eye128 = consts.tile([128, 128], F32, tag="eye128")
        ones1 = consts.tile([1, 128], F32R, tag="ones1")
        bias_sb = consts.tile([1, 3 * H], F32R, tag="bias")
        wh_sb = consts.tile([128, 4, 3 * H], F32R, tag="wh")
        nc.sync.dma_start(eye8[:], eye8_d[:])
        nc.sync.dma_start(eye128[:], eye128_d[:])
        nc.sync.dma_start(ones1[:], ones1_d[:])
        nc.sync.dma_start(bias_sb[:], bias_d[:])
        nc.sync.dma_start(wh_sb[:], wh_d.rearrange("(c p) n -> p c n", p=128))

        # ---------------- PRE phase ----------------
        with ExitStack() as c1, nc.named_scope("pre"):
            wpre = c1.enter_context(tc.tile_pool(name="wpre", bufs=1))
            wx_sb = wpre.tile([128, 4, 3 * H], F32R, tag="wx")
            wp_sb = wpre.tile([128, 4, 3 * H], F32R, tag="wp")
            nc.sync.dma_start(wx_sb[:], wx_d.rearrange("(c p) n -> p c n", p=128))
            nc.sync.dma_start(wp_sb[:], wp_d.rearrange("(c p) n -> p c n", p=128))

            pre_in = c1.enter_context(tc.tile_pool(name="pre_in", bufs=3))
            pre_sb = c1.enter_context(tc.tile_pool(name="pre_sb", bufs=2))
            pre_ps = c1.enter_context(tc.tile_pool(name="pre_ps", bufs=1, space="PSUM"))

            def pre_block(bi):
                """bi: block index (16 timesteps), scalar expr or int."""
                xb = pre_in.tile([128, D], F32, tag="xb")
                cb = pre_in.tile([128, H], F32, tag="cb")
                nc.sync.dma_start(
                    xb[:], x_d[:, ds(bi * 16, 16), :].rearrange("b t d -> t b d"))
                nc.sync.dma_start(
                    cb[:], ctx_d[:, ds(bi * 16, 16), :].rearrange("b t d -> t b d"))
                pTx = pre_ps.tile([128, D], F32, tag="pTx")
                pTc = pre_ps.tile([128, H], F32, tag="pTc")
                for c in range(4):
                    nc.tensor.transpose(
                        pTx[:, c * 128:(c + 1) * 128],
                        xb[:, c * 128:(c + 1) * 128], eye128[:])
                for c in range(4):
                    nc.tensor.transpose(
                        pTc[:, c * 128:(c + 1) * 128],
                        cb[:, c * 128:(c + 1) * 128], eye128[:])
                xT = pre_sb.tile([128, D], F32R, tag="xT")
                cT = pre_sb.tile([128, H], F32R, tag="cT")
                nc.scalar.activation(xT[:], pTx[:], AF.Identity)
                nc.vector.tensor_copy(cT[:], pTc[:])
                asb = pre_sb.tile([128, 3, H], F32R, tag="asb")
                for g in range(3):
                    pz = pre_ps.tile([128, H], F32, tag=f"pz{g}")
                    nc.tensor.matmul(
                        pz[:], ones1[:], bias_sb[:, g * H:(g + 1) * H],
                        start=True, stop=False)
                    for c in range(4):
                        nc.tensor.matmul(
                            pz[:], xT[:, c * 128:(c + 1) * 128],
                            wx_sb[:, c, g * H:(g + 1) * H],
                            start=False, stop=False)
                    for c in range(4):
                        nc.tensor.matmul(
                            pz[:], cT[:, c * 128:(c + 1) * 128],
                            wp_sb[:, c, g * H:(g + 1) * H],
                            start=False, stop=(c == 3))
                    if g == 0:
                        nc.scalar.activation(asb[:, g, :], pz[:], AF.Identity)
                    else:
                        nc.vector.tensor_copy(asb[:, g, :], pz[:])
                nc.scalar.dma_start(ar_d[ds(bi * 16, 16), :, :, :], asb[:])

            with tc.For_i(0, nblk // 2, 1, staggered_reset=True) as ib:
                pre_block(ib * 2)
                pre_block(ib * 2 + 1)

        # ---------------- SCAN phase ----------------
        with ExitStack() as c2, nc.named_scope("scan"):
            spool = c2.enter_context(tc.tile_pool(name="scan_sb", bufs=1))
            spool2 = c2.enter_context(tc.tile_pool(name="scan_sb2", bufs=2))
            sps = c2.enter_context(tc.tile_pool(name="scan_ps", bufs=2, space="PSUM"))
            sps1 = c2.enter_context(tc.tile_pool(name="scan_ps1", bufs=1, space="PSUM"))

            # h staging: slot k holds h after step (8m + k), transposed layout
            hstage = spool.tile([128, RING * 4 * BC], F32R, tag="hstage")

            def hslot(k):
                k = k % RING
                return hstage[:, k * 32:(k + 1) * 32]

            # ar staging for RING steps: [b, k, g, H]
            art_all = spool.tile([BC, RING, 3, H], F32R, tag="art_all")

            # init h into slot RING-1 (read by step 0)
            h0_sb = spool.tile([BC, H], F32, tag="h0")
            nc.sync.dma_start(h0_sb[:], h0_d[:])
            pT0 = sps1.tile([128, 96], F32, tag="pT")
            for c in range(4):
                nc.tensor.transpose(
                    pT0[:, c * 8:(c + 1) * 8], h0_sb[:, c * 128:(c + 1) * 128],
                    eye8[:].bitcast(F32))
            nc.vector.tensor_copy(hslot(RING - 1), pT0[:, 0:32])

            # preload ar for steps 0..RING-1
            nc.sync.dma_start(
                art_all[:], ar_d[0:RING, :, :, :].rearrange("t b g n -> b t g n"))

            def gru_step(kk):
                h_in = hslot(kk - 1)
                h_out = hslot(kk)
                pzr = sps.tile([BC, H], F32, tag="pzr")
                pzi = sps.tile([BC, H], F32, tag="pzi")
                pzn = sps.tile([BC, H], F32, tag="pzn")
                pT = sps1.tile([128, 96], F32, tag="pT")

                # zr = a_r[t] + h @ W_hr.T ; zi likewise (back to back so PE
                # runs the zi matmuls while ACT/DVE work on the r path)
                nc.tensor.matmul(pzr[:], eye8[:], art_all[:, kk, 0, :],
                                 start=True, stop=False)
                for c in range(4):
                    nc.tensor.matmul(
                        pzr[:], h_in[:, c * 8:(c + 1) * 8],
                        wh_sb[:, c, 0:H], start=False, stop=(c == 3))
                nc.tensor.matmul(pzi[:], eye8[:], art_all[:, kk, 1, :],
                                 start=True, stop=False)
                for c in range(4):
                    nc.tensor.matmul(
                        pzi[:], h_in[:, c * 8:(c + 1) * 8],
                        wh_sb[:, c, H:2 * H], start=False, stop=(c == 3))

                r_sb = spool2.tile([BC, H], F32, tag="r_sb")
                nc.scalar.activation(r_sb[:], pzr[:], AF.Sigmoid)
                for c in range(4):
                    nc.tensor.transpose(
                        pT[:, c * 8:(c + 1) * 8],
                        r_sb[:, c * 128:(c + 1) * 128], eye8[:].bitcast(F32))
                rh = spool2.tile([128, 4 * BC], F32R, tag="rh")
                nc.vector.tensor_mul(rh[:], pT[:, 0:32], h_in.bitcast(F32))

                # zn = a_n[t] + (r*h) @ W_hn.T
                nc.tensor.matmul(pzn[:], eye8[:], art_all[:, kk, 2, :],
                                 start=True, stop=False)
                for c in range(4):
                    nc.tensor.matmul(
                        pzn[:], rh[:, c * 8:(c + 1) * 8],
                        wh_sb[:, c, 2 * H:3 * H], start=False, stop=(c == 3))

                # i path (overlaps the zn matmuls)
                i_sb = spool2.tile([BC, H], F32, tag="i_sb")
                nc.scalar.activation(i_sb[:], pzi[:], AF.Sigmoid)
                for c in range(4):
                    nc.tensor.transpose(
                        pT[:, 32 + c * 8:32 + (c + 1) * 8],
                        i_sb[:, c * 128:(c + 1) * 128], eye8[:].bitcast(F32))
                jT = spool2.tile([128, 4 * BC], F32, tag="jT")
                nc.vector.tensor_scalar(
                    jT[:], pT[:, 32:64], -1.0, 1.0, ALU.mult, ALU.add)
                iT = spool2.tile([128, 4 * BC], F32, tag="iT")
                nc.scalar.activation(iT[:], pT[:, 32:64], AF.Identity)
                a_sb = spool2.tile([128, 4 * BC], F32, tag="a_sb")
                nc.vector.tensor_mul(a_sb[:], jT[:], h_in.bitcast(F32))

                n_sb = spool2.tile([BC, H], F32, tag="n_sb")
                nc.scalar.activation(n_sb[:], pzn[:], AF.Tanh)
                for c in range(4):
                    nc.tensor.transpose(
                        pT[:, 64 + c * 8:64 + (c + 1) * 8],
                        n_sb[:, c * 128:(c + 1) * 128], eye8[:].bitcast(F32))
                # h' = (1-i)*h + i*n
                b_sb = spool2.tile([128, 4 * BC], F32, tag="b_sb")
                nc.vector.tensor_mul(b_sb[:], iT[:], pT[:, 64:96])
                nc.vector.tensor_add(h_out, a_sb[:], b_sb[:])

            def flush_hist(t_off, half):
                """DMA hstage slots [half*4, half*4+4) -> hist[t_off .. +4)."""
                nc.sync.dma_start(
                    hist_d[ds(t_off, 4), :, :].rearrange("t p c -> p t c"),
                    hstage[:, half * 128:(half + 1) * 128])

            def refill(t_off, half):
                nc.scalar.dma_start(
                    art_all[:, half * 4:(half + 1) * 4, :, :],
                    ar_d[ds(t_off, 4), :, :, :].rearrange("t b g n -> b t g n"))

            with tc.For_i(0, t_steps // RING - 1, 1, staggered_reset=True,
                          hint_engines=(mybir.EngineType.PE,)) as i:
                for k in range(RING):
                    gru_step(k)
                    if k == 3:
                        flush_hist(i * RING, 0)
                        refill((i + 1) * RING, 0)
                    elif k == 7:
                        flush_hist(i * RING + 4, 1)
                        refill((i + 1) * RING + 4, 1)

            # static epilogue: last RING steps, no refill
            te = t_steps - RING
            for k in range(RING):
                gru_step(k)
                if k == 3:
                    nc.sync.dma_start(
                        hist_d[te:te + 4, :, :].rearrange("t p c -> p t c"),
                        hstage[:, 0:128])
                elif k == 7:
                    nc.sync.dma_start(
                        hist_d[te + 4:te + 8, :, :].rearrange("t p c -> p t c"),
                        hstage[:, 128:256])

    nc.finalize()
    return nc


def _prep_shared(W_ir, W_hr, W_pr, W_ii, W_hi, W_pi, W_in, W_hn, W_pn,
                 b_ir, b_hr, b_pr, b_ii, b_hi, b_pi, b_in, b_hn, b_pn):
    f = np.float32
    wxT = np.ascontiguousarray(
        np.concatenate([W_ir.T, W_ii.T, W_in.T], axis=1), dtype=f)
    wpT = np.ascontiguousarray(
        np.concatenate([W_pr.T, W_pi.T, W_pn.T], axis=1), dtype=f)
    whT = np.ascontiguousarray(
        np.concatenate([W_hr.T, W_hi.T, W_hn.T], axis=1), dtype=f)
    bias = np.concatenate([
        b_ir + b_hr + b_pr, b_ii + b_hi + b_pi, b_in + b_hn + b_pn
    ]).astype(f).reshape(1, 3 * H)
    return {
        "wxT": wxT, "wpT": wpT, "whT": whT, "bias": bias,
        "eye8": np.eye(8, dtype=f),
        "eye128": np.eye(128, dtype=f),
        "ones1": np.ones((1, 128), dtype=f),
    }


def kernel(x, h0, ctx, W_ir, b_ir, W_hr, b_hr, W_pr, b_pr,
           W_ii, b_ii, W_hi, b_hi, W_pi, b_pi,
           W_in, b_in, W_hn, b_hn, W_pn, b_pn, _t_steps=T, _trace=False):
    x = np.asarray(x, np.float32)
    ctx = np.asarray(ctx, np.float32)
    h0 = np.asarray(h0, np.float32)

    if _t_steps not in _CACHE:
        _CACHE[_t_steps] = build_program(_t_steps)
    nc = _CACHE[_t_steps]

    shared = _prep_shared(
        np.asarray(W_ir, np.float32), np.asarray(W_hr, np.float32),
        np.asarray(W_pr, np.float32), np.asarray(W_ii, np.float32),
        np.asarray(W_hi, np.float32), np.asarray(W_pi, np.float32),
        np.asarray(W_in, np.float32), np.asarray(W_hn, np.float32),
        np.asarray(W_pn, np.float32),
        np.asarray(b_ir, np.float32), np.asarray(b_hr, np.float32),
        np.asarray(b_pr, np.float32), np.asarray(b_ii, np.float32),
        np.asarray(b_hi, np.float32), np.asarray(b_pi, np.float32),
        np.asarray(b_in, np.float32), np.asarray(b_hn, np.float32),
        np.asarray(b_pn, np.float32))

    in_maps = []
    for c in range(NCORES):
        sl = slice(c * BC, (c + 1) * BC)
        m = dict(shared)
        m["x"] = np.ascontiguousarray(x[sl])
        m["ctx"] = np.ascontiguousarray(ctx[sl])
        m["h0"] = np.ascontiguousarray(h0[sl])
        in_maps.append(m)

    res = run_bass_kernel_spmd(nc, in_maps, list(range(NCORES)), trace=_trace)

    tt = _t_steps
    ys = np.empty((B, tt, H), np.float32)
    for c in range(NCORES):
        hist = res.results[c]["hist"].reshape(tt, 128, 4, BC)
        # hist[t, p, cc, b] = h_t[b, cc*128+p]
        ys[c * BC:(c + 1) * BC] = np.ascontiguousarray(
            hist.transpose(3, 0, 2, 1)).reshape(BC, tt, H)
    hT = np.ascontiguousarray(ys[:, -1, :])
    if _trace:
        return (ys, hT), res
    return ys, hT


# revision 19
# speedup vs baseline: 2.4774x; 2.4774x over previous
"""GRU-with-peephole (NewGRU) Trainium2 kernel.

B=64, T=512, D=H=512. Data-parallel over batch: 8 cores x 8 batch each.

Phase PRE: a_g[t,b,:] = x[b,t]@W_ig.T + ctx[b,t]@W_pg.T + (b_ig+b_hg+b_pg)
           for g in {r,i,n}, staged to DRAM as [t, b, g, H].
Phase SCAN (sequential over T):
    zr = a_r[t] + h @ W_hr.T          r = sigmoid(zr)
    zi = a_i[t] + h @ W_hi.T          i = sigmoid(zi)
    zn = a_n[t] + (r*h) @ W_hn.T      n = tanh(zn)
    h  = (1-i)*h + i*n
    hist[t] = h  in transposed layout [128(p), 4(c)*8(b)] = h[b, c*128+p]
Host reassembles ys[b,t,h] from hist.

Matmuls are float32r (TF32-like, 1 cyc/row at N=512) with fp32 PSUM
accumulation. The scan keeps h transposed (hT [128, 32]) in a rotating
8-slot SBUF staging tile whose slices are both the matmul inputs and
the DMA-out source; gate matmuls are h-stationary (lhsT = hT chunk
[128,8], rhs = W^T chunk [128,512] moving). Per-step projections are
injected into PSUM via an eye(8) matmul with start=True, which also
initializes the accumulation group.
"""

import sys

if "/opt/trn_rl_repo" not in sys.path:
    sys.path.insert(0, "/opt/trn_rl_repo")

from contextlib import ExitStack

import numpy as np

import concourse.bass as bass
import concourse.bacc as bacc
import concourse.mybir as mybir
import concourse.tile as tile
from concourse.bass_utils import run_bass_kernel_spmd

B, T, D, H = 64, 512, 512, 512
NCORES = 8
BC = B // NCORES  # 8 batch per core
RING = 8          # steps per scan loop body / h+ar staging slots

F32 = mybir.dt.float32
F32R = mybir.dt.float32r
AF = mybir.ActivationFunctionType
ALU = mybir.AluOpType
ds = bass.ds

_CACHE = {}


def r32(ap):
    return ap.bitcast(F32R)


def build_program(t_steps=T, unroll_all=False, pt_bufs=1, staggered=True):
    """Build the SPMD Bass program for one core (same program on all 8)."""
    assert t_steps % 32 == 0
    nblk = t_steps // 16  # PRE blocks of 16 timesteps
    nc = bacc.Bacc("TRN2", target_bir_lowering=False)

    x_d = nc.declare_dram_parameter("x", [BC, t_steps, D], F32, isOutput=False)
    ctx_d = nc.declare_dram_parameter("ctx", [BC, t_steps, H], F32, isOutput=False)
    h0_d = nc.declare_dram_parameter("h0", [BC, H], F32, isOutput=False)
    wx_d = nc.declare_dram_parameter("wxT", [D, 3 * H], F32R, isOutput=False)
    wp_d = nc.declare_dram_parameter("wpT", [H, 3 * H], F32R, isOutput=False)
    wh_d = nc.declare_dram_parameter("whT", [H, 3 * H], F32R, isOutput=False)
    bias_d = nc.declare_dram_parameter("bias", [1, 3 * H], F32R, isOutput=False)
    eye8_d = nc.declare_dram_parameter("eye8", [8, 8], F32R, isOutput=False)
    eye128_d = nc.declare_dram_parameter("eye128", [128, 128], F32, isOutput=False)
    ones1_d = nc.declare_dram_parameter("ones1", [1, 128], F32R, isOutput=False)
    hist_d = nc.declare_dram_parameter(
        "hist", [t_steps, 128, 4 * BC], F32R, isOutput=True)
    ar_d = nc.dram_tensor("ar_stage", [t_steps, BC, 3, H], F32R)

    with tile.TileContext(nc) as tc, ExitStack() as c0:
        # ---------------- persistent constants / weights ----------------
        consts = c0.enter_context(tc.tile_pool(name="consts", bufs=1))
        eye8 = consts.tile([8, 8], F32R, tag="eye8")
        eye128 = consts.tile([128, 128], F32, tag="eye128")
        ones1 = consts.tile([1, 128], F32R, tag="ones1")
        bias_sb = consts.tile([1, 3 * H], F32R, tag="bias")
        wh_sb = consts.tile([128, 4, 3 * H], F32R, tag="wh")
        nc.sync.dma_start(eye8[:], eye8_d[:])
        nc.sync.dma_start(eye128[:], eye128_d[:])
        nc.sync.dma_start(ones1[:], ones1_d[:])
        nc.sync.dma_start(bias_sb[:], bias_d[:])
        nc.sync.dma_start(wh_sb[:], wh_d.rearrange("(c p) n -> p c n", p=128))
        # prime the ACT table set (sigmoid_and_others) once, before any loop
        warm = consts.tile([1, 8], F32, tag="actwarm")
        nc.scalar.activation(warm[:], eye8[0:1, :].bitcast(F32), AF.Sigmoid)

        # ---------------- PRE phase ----------------
        with ExitStack() as c1, nc.named_scope("pre"):
            wpre = c1.enter_context(tc.tile_pool(name="wpre", bufs=1))
            wx_sb = wpre.tile([128, 4, 3 * H], F32R, tag="wx")
            wp_sb = wpre.tile([128, 4, 3 * H], F32R, tag="wp")
            nc.sync.dma_start(wx_sb[:], wx_d.rearrange("(c p) n -> p c n", p=128))
            nc.sync.dma_start(wp_sb[:], wp_d.rearrange("(c p) n -> p c n", p=128))

            pre_in = c1.enter_context(tc.tile_pool(name="pre_in", bufs=3))
            pre_sb = c1.enter_context(tc.tile_pool(name="pre_sb", bufs=2))
            pre_ps = c1.enter_context(tc.tile_pool(name="pre_ps", bufs=1, space="PSUM"))

            def pre_block(bi):
                """bi: block index (16 timesteps), scalar expr or int."""
                xb = pre_in.tile([128, D], F32, tag="xb")
                cb = pre_in.tile([128, H], F32, tag="cb")
                nc.sync.dma_start(
                    xb[:], x_d[:, ds(bi * 16, 16), :].rearrange("b t d -> t b d"))
                nc.sync.dma_start(
                    cb[:], ctx_d[:, ds(bi * 16, 16), :].rearrange("b t d -> t b d"))
                pTx = pre_ps.tile([128, D], F32, tag="pTx")
                pTc = pre_ps.tile([128, H], F32, tag="pTc")
                for c in range(4):
                    nc.tensor.transpose(
                        pTx[:, c * 128:(c + 1) * 128],
                        xb[:, c * 128:(c + 1) * 128], eye128[:])
                for c in range(4):
                    nc.tensor.transpose(
                        pTc[:, c * 128:(c + 1) * 128],
                        cb[:, c * 128:(c + 1) * 128], eye128[:])
                xT = pre_sb.tile([128, D], F32R, tag="xT")
                cT = pre_sb.tile([128, H], F32R, tag="cT")
                nc.vector.tensor_copy(xT[:], pTx[:])
                nc.vector.tensor_copy(cT[:], pTc[:])
                asb = pre_sb.tile([128, 3, H], F32R, tag="asb")
                for g in range(3):
                    pz = pre_ps.tile([128, H], F32, tag=f"pz{g}")
                    nc.tensor.matmul(
                        pz[:], ones1[:], bias_sb[:, g * H:(g + 1) * H],
                        start=True, stop=False)
                    for c in range(4):
                        nc.tensor.matmul(
                            pz[:], xT[:, c * 128:(c + 1) * 128],
                            wx_sb[:, c, g * H:(g + 1) * H],
                            start=False, stop=False)
                    for c in range(4):
                        nc.tensor.matmul(
                            pz[:], cT[:, c * 128:(c + 1) * 128],
                            wp_sb[:, c, g * H:(g + 1) * H],
                            start=False, stop=(c == 3))
                    nc.vector.tensor_copy(asb[:, g, :], pz[:])
                nc.scalar.dma_start(ar_d[ds(bi * 16, 16), :, :, :], asb[:])

            with tc.For_i(0, nblk // 2, 1, staggered_reset=True) as ib:
                pre_block(ib * 2)
                pre_block(ib * 2 + 1)

        # ---------------- SCAN phase ----------------
        with ExitStack() as c2, nc.named_scope("scan"):
            spool = c2.enter_context(tc.tile_pool(name="scan_sb", bufs=1))
            spool2 = c2.enter_context(tc.tile_pool(name="scan_sb2", bufs=2))
            sps = c2.enter_context(tc.tile_pool(name="scan_ps", bufs=2, space="PSUM"))
            sps1 = c2.enter_context(tc.tile_pool(name="scan_ps1", bufs=pt_bufs, space="PSUM"))

            # h staging: slot k holds h after step (8m + k), transposed layout
            hstage = spool.tile([128, RING * 4 * BC], F32R, tag="hstage")

            def hslot(k):
                k = k % RING
                return hstage[:, k * 32:(k + 1) * 32]

            # ar staging for RING steps: [b, k, g, H]
            art_all = spool.tile([BC, RING, 3, H], F32R, tag="art_all")

            # init h into slot RING-1 (read by step 0)
            h0_sb = spool.tile([BC, H], F32, tag="h0")
            nc.sync.dma_start(h0_sb[:], h0_d[:])
            pT0 = sps1.tile([128, 96], F32, tag="pT")
            for c in range(4):
                nc.tensor.transpose(
                    pT0[:, c * 8:(c + 1) * 8], h0_sb[:, c * 128:(c + 1) * 128],
                    eye8[:].bitcast(F32))
            nc.vector.tensor_copy(hslot(RING - 1), pT0[:, 0:32])

            # preload ar for steps 0..RING-1
            nc.sync.dma_start(
                art_all[:], ar_d[0:RING, :, :, :].rearrange("t b g n -> b t g n"))

            def gru_step(kk):
                h_in = hslot(kk - 1)
                h_out = hslot(kk)
                pzr = sps.tile([BC, H], F32, tag="pzr")
                pzi = sps.tile([BC, H], F32, tag="pzi")
                pzn = sps.tile([BC, H], F32, tag="pzn")
                pT = sps1.tile([128, 96], F32, tag="pT")

                # zr = a_r[t] + h @ W_hr.T ; zi likewise (back to back so PE
                # runs the zi matmuls while ACT/DVE work on the r path)
                nc.tensor.matmul(pzr[:], eye8[:], art_all[:, kk, 0, :],
                                 start=True, stop=False)
                for c in range(4):
                    nc.tensor.matmul(
                        pzr[:], h_in[:, c * 8:(c + 1) * 8],
                        wh_sb[:, c, 0:H], start=False, stop=(c == 3))
                nc.tensor.matmul(pzi[:], eye8[:], art_all[:, kk, 1, :],
                                 start=True, stop=False)
                for c in range(4):
                    nc.tensor.matmul(
                        pzi[:], h_in[:, c * 8:(c + 1) * 8],
                        wh_sb[:, c, H:2 * H], start=False, stop=(c == 3))

                r_sb = spool2.tile([BC, H], F32, tag="r_sb")
                nc.scalar.activation(r_sb[:], pzr[:], AF.Sigmoid)
                for c in range(4):
                    nc.tensor.transpose(
                        pT[:, c * 8:(c + 1) * 8],
                        r_sb[:, c * 128:(c + 1) * 128], eye8[:].bitcast(F32))
                rh = spool2.tile([128, 4 * BC], F32R, tag="rh")
                nc.vector.tensor_mul(rh[:], pT[:, 0:32], h_in.bitcast(F32))

                # zn = a_n[t] + (r*h) @ W_hn.T
                nc.tensor.matmul(pzn[:], eye8[:], art_all[:, kk, 2, :],
                                 start=True, stop=False)
                for c in range(4):
                    nc.tensor.matmul(
                        pzn[:], rh[:, c * 8:(c + 1) * 8],
                        wh_sb[:, c, 2 * H:3 * H], start=False, stop=(c == 3))

                # i path (overlaps the zn matmuls)
                i_sb = spool2.tile([BC, H], F32, tag="i_sb")
                nc.scalar.activation(i_sb[:], pzi[:], AF.Sigmoid)
                for c in range(4):
                    nc.tensor.transpose(
                        pT[:, 32 + c * 8:32 + (c + 1) * 8],
                        i_sb[:, c * 128:(c + 1) * 128], eye8[:].bitcast(F32))
                jT = spool2.tile([128, 4 * BC], F32, tag="jT")
                nc.vector.tensor_scalar(
                    jT[:], pT[:, 32:64], -1.0, 1.0, ALU.mult, ALU.add)
                iT = spool2.tile([128, 4 * BC], F32, tag="iT")
                nc.vector.tensor_copy(iT[:], pT[:, 32:64])
                a_sb = spool2.tile([128, 4 * BC], F32, tag="a_sb")
                nc.vector.tensor_mul(a_sb[:], jT[:], h_in.bitcast(F32))

                n_sb = spool2.tile([BC, H], F32, tag="n_sb")
                nc.scalar.activation(n_sb[:], pzn[:], AF.Tanh)
                for c in range(4):
                    nc.tensor.transpose(
                        pT[:, 64 + c * 8:64 + (c + 1) * 8],
                        n_sb[:, c * 128:(c + 1) * 128], eye8[:].bitcast(F32))
                # h' = (1-i)*h + i*n
                b_sb = spool2.tile([128, 4 * BC], F32, tag="b_sb")
                nc.vector.tensor_mul(b_sb[:], iT[:], pT[:, 64:96])
                nc.vector.tensor_add(h_out, a_sb[:], b_sb[:])

            def flush_hist(t_off, half):
                """DMA hstage slots [half*4, half*4+4) -> hist[t_off .. +4)."""
                nc.sync.dma_start(
                    hist_d[ds(t_off, 4), :, :].rearrange("t p c -> p t c"),
                    hstage[:, half * 128:(half + 1) * 128])

            def refill(t_off, half):
                nc.scalar.dma_start(
                    art_all[:, half * 4:(half + 1) * 4, :, :],
                    ar_d[ds(t_off, 4), :, :, :].rearrange("t b g n -> b t g n"))

            def loop_body(i):
                for k in range(RING):
                    gru_step(k)
                    if k == 3:
                        flush_hist(i * RING, 0)
                        refill((i + 1) * RING, 0)
                    elif k == 7:
                        flush_hist(i * RING + 4, 1)
                        refill((i + 1) * RING + 4, 1)

            if unroll_all:
                for i in range(t_steps // RING - 1):
                    loop_body(i)
            else:
                with tc.For_i(0, t_steps // RING - 1, 1,
                              staggered_reset=staggered,
                              hint_engines=(mybir.EngineType.PE,)) as i:
                    loop_body(i)

            # static epilogue: last RING steps, no refill
            te = t_steps - RING
            for k in range(RING):
                gru_step(k)
                if k == 3:
                    nc.sync.dma_start(
                        hist_d[te:te + 4, :, :].rearrange("t p c -> p t c"),
                        hstage[:, 0:128])
                elif k == 7:
                    nc.sync.dma_start(
                        hist_d[te + 4:te + 8, :, :].rearrange("t p c -> p t c"),
                        hstage[:, 128:256])

    nc.finalize()
    return nc


def _prep_shared(W_ir, W_hr, W_pr, W_ii, W_hi, W_pi, W_in, W_hn, W_pn,
                 b_ir, b_hr, b_pr, b_ii, b_hi, b_pi, b_in, b_hn, b_pn):
    f = np.float32
    wxT = np.ascontiguousarray(
        np.concatenate([W_ir.T, W_ii.T, W_in.T], axis=1), dtype=f)
    wpT = np.ascontiguousarray(
        np.concatenate([W_pr.T, W_pi.T, W_pn.T], axis=1), dtype=f)
    whT = np.ascontiguousarray(
        np.concatenate([W_hr.T, W_hi.T, W_hn.T], axis=1), dtype=f)
    bias = np.concatenate([
        b_ir + b_hr + b_pr, b_ii + b_hi + b_pi, b_in + b_hn + b_pn
    ]).astype(f).reshape(1, 3 * H)
    return {
        "wxT": wxT, "wpT": wpT, "whT": whT, "bias": bias,
        "eye8": np.eye(8, dtype=f),
        "eye128": np.eye(128, dtype=f),
        "ones1": np.ones((1, 128), dtype=f),
    }


def kernel(x, h0, ctx, W_ir, b_ir, W_hr, b_hr, W_pr, b_pr,
           W_ii, b_ii, W_hi, b_hi, W_pi, b_pi,
           W_in, b_in, W_hn, b_hn, W_pn, b_pn, _t_steps=T, _trace=False):
    x = np.asarray(x, np.float32)
    ctx = np.asarray(ctx, np.float32)
    h0 = np.asarray(h0, np.float32)

    if _t_steps not in _CACHE:
        _CACHE[_t_steps] = build_program(_t_steps, unroll_all=True)
    nc = _CACHE[_t_steps]

    shared = _prep_shared(
        np.asarray(W_ir, np.float32), np.asarray(W_hr, np.float32),
        np.asarray(W_pr, np.float32), np.asarray(W_ii, np.float32),
        np.asarray(W_hi, np.float32), np.asarray(W_pi, np.float32),
        np.asarray(W_in, np.float32), np.asarray(W_hn, np.float32),
        np.asarray(W_pn, np.float32),
        np.asarray(b_ir, np.float32), np.asarray(b_hr, np.float32),
        np.asarray(b_pr, np.float32), np.asarray(b_ii, np.float32),
        np.asarray(b_hi, np.float32), np.asarray(b_pi, np.float32),
        np.asarray(b_in, np.float32), np.asarray(b_hn, np.float32),
        np.asarray(b_pn, np.float32))

    in_maps = []
    for c in range(NCORES):
        sl = slice(c * BC, (c + 1) * BC)
        m = dict(shared)
        m["x"] = np.ascontiguousarray(x[sl])
        m["ctx"] = np.ascontiguousarray(ctx[sl])
        m["h0"] = np.ascontiguousarray(h0[sl])
        in_maps.append(m)

    res = run_bass_kernel_spmd(nc, in_maps, list(range(NCORES)), trace=_trace)

    tt = _t_steps
    ys = np.empty((B, tt, H), np.float32)
    for c in range(NCORES):
        hist = res.results[c]["hist"].reshape(tt, 128, 4, BC)
        # hist[t, p, cc, b] = h_t[b, cc*128+p]
        ys[c * BC:(c + 1) * BC] = np.ascontiguousarray(
            hist.transpose(3, 0, 2, 1)).reshape(BC, tt, H)
    hT = np.ascontiguousarray(ys[:, -1, :])
    if _trace:
        return (ys, hT), res
    return ys, hT
